# revision 6
# baseline (speedup 1.0000x reference)
"""Self-attention scores kernel for Trainium2, 8-core SPMD.

Computes softmax((x@Wq+bq) @ (x@Wq+bq)^T / sqrt(64)) per head
(reference reuses the query projection for k, bug-for-bug).

Sharding: 32 (batch, head) pairs split 4-per-core across 8 cores.
Core c handles batch c//4, heads 4*(c%4) .. 4*(c%4)+3.
Each core gets x[b]^T (host-transposed), its Wq column slice, and its
bias slice; it computes q^T = Wq_slice^T @ x^T (+bias), then per head
the [2048, 2048] score block + row softmax, streaming 1 MiB row-blocks
back to HBM.
"""

import numpy as np

import concourse.bass as bass
import concourse.mybir as mybir
import concourse.tile as tile
from concourse import bacc
from concourse.bass_utils import run_bass_kernel_spmd

B = 2
S = 2048
D = 1024
H = 16
HS = 64
N_CORES = 8
HEADS_PER_CORE = 4  # 2 pairs of 2 heads (pair = 128 partitions)
KK = D // 128  # 8 k-tiles for the projection contraction
NQ = S // 128  # 16 q row-blocks per head
NC_ = S // 512  # 4 key chunks of 512

# Matmul input dtype: float32 is exact but 4 cycles/row on the PE;
# float32r runs at full rate for N>=256 with relaxed (tf32-like)
# precision. The BIR verifier requires fp32r matmul operands to be
# *produced* as fp32r, so the input DRAM tensors and SBUF tiles feeding
# the PE are declared float32r (numpy binding is still float32).
MM_DT = mybir.dt.float32r

F32 = mybir.dt.float32


def _mm_view(ap):
    return ap


def _build():
    nc = bacc.Bacc("TRN2", target_bir_lowering=False, debug=False)
    xT = nc.dram_tensor("xT", [D, S], MM_DT, kind="ExternalInput").ap()
    WqS = nc.dram_tensor("WqS", [D, HEADS_PER_CORE * HS], MM_DT, kind="ExternalInput").ap()
    bqS = nc.dram_tensor("bqS", [128, 2], F32, kind="ExternalInput").ap()
    out = nc.dram_tensor("out", [HEADS_PER_CORE, S, S], F32, kind="ExternalOutput").ap()

    with tile.TileContext(nc) as tc:
        with (
            tc.tile_pool(name="consts", bufs=1) as consts,
            tc.tile_pool(name="qt", bufs=2) as qt_pool,
            tc.tile_pool(name="xt", bufs=KK) as xt_pool,
            tc.tile_pool(name="ps", bufs=2, space="PSUM") as ps_pool,
            tc.tile_pool(name="et", bufs=6) as et_pool,
            tc.tile_pool(name="small", bufs=8) as small,
        ):
            w = consts.tile([128, KK, HEADS_PER_CORE * HS], MM_DT)
            nc.sync.dma_start(out=w[:], in_=WqS.rearrange("(kk p) c -> p kk c", p=128))
            bias = consts.tile([128, 2], F32)
            nc.sync.dma_start(out=bias[:], in_=bqS)

            # x^T streamed as 8 independent k-tiles so projection matmuls
            # can start as soon as each tile lands.
            xts = []
            for kk in range(KK):
                xtt = xt_pool.tile([128, S], MM_DT, tag="xt")
                nc.sync.dma_start(out=xtt[:], in_=xT[kk * 128 : (kk + 1) * 128, :])
                xts.append(xtt)

            # ---- Projection for one head-pair ----
            def project(g):
                qtg = qt_pool.tile([128, S], MM_DT, tag="qt")
                ps = ps_pool.tile([128, S], F32, tag="ps")
                for n in range(NC_):
                    for kk in range(KK):
                        nc.tensor.matmul(
                            ps[:, n * 512 : (n + 1) * 512],
                            lhsT=w[:, kk, g * 128 : (g + 1) * 128],
                            rhs=xts[kk][:, n * 512 : (n + 1) * 512],
                            start=(kk == 0),
                            stop=(kk == KK - 1),
                        )
                    nc.scalar.activation(
                        out=qtg[:, n * 512 : (n + 1) * 512],
                        in_=ps[:, n * 512 : (n + 1) * 512],
                        func=mybir.ActivationFunctionType.Identity,
                        bias=bias[:, g : g + 1],
                        scale=1.0,
                    )
                return qtg

            # ---- Scores + softmax for one head, streamed per row-block ----
            def score_head(h, qtg):
                pb = (h % 2) * 64
                for i in range(NQ):
                    ps = ps_pool.tile([128, S], F32, tag="ps")
                    lhsT = qtg[pb : pb + 64, i * 128 : (i + 1) * 128]
                    for j in range(NC_):
                        nc.tensor.matmul(
                            ps[:, j * 512 : (j + 1) * 512],
                            lhsT=lhsT,
                            rhs=qtg[pb : pb + 64, j * 512 : (j + 1) * 512],
                            start=True,
                            stop=True,
                        )
                    et = et_pool.tile([128, S], F32, tag="et")
                    sums = small.tile([128, 1], F32, tag="sm")
                    nc.scalar.activation(
                        out=et[:],
                        in_=ps[:],
                        func=mybir.ActivationFunctionType.Exp,
                        scale=1.0 / np.sqrt(float(HS)),
                        accum_out=sums[:],
                    )
                    recip = small.tile([128, 1], F32, tag="rc")
                    nc.vector.reciprocal(recip[:], sums[:])
                    nc.vector.tensor_scalar_mul(et[:], et[:], recip[:])
                    nc.sync.dma_start(
                        out=out[h, i * 128 : (i + 1) * 128, :], in_=et[:]
                    )

            # Emission order sets Tile's scheduling priority: get pair-0's
            # output stream going first; pair-1's projection then fills PE
            # idle slots during streaming.
            qt0 = project(0)
            score_head(0, qt0)
            score_head(1, qt0)
            qt1 = project(1)
            score_head(2, qt1)
            score_head(3, qt1)
    nc.compile()
    return nc


_NC_CACHE = None


def kernel(x, Wq, bq):
    global _NC_CACHE
    x = np.asarray(x, dtype=np.float32)
    Wq = np.asarray(Wq, dtype=np.float32)
    bq = np.asarray(bq, dtype=np.float32)
    assert x.shape == (B, S, D) and Wq.shape == (D, D) and bq.shape == (D,)

    if _NC_CACHE is None:
        _NC_CACHE = _build()
    nc = _NC_CACHE

    xTs = [np.ascontiguousarray(x[b].T) for b in range(B)]
    in_maps = []
    for c in range(N_CORES):
        b, hg = divmod(c, N_CORES // B)
        h0 = hg * HEADS_PER_CORE
        in_maps.append(
            {
                "xT": xTs[b],
                "WqS": np.ascontiguousarray(Wq[:, h0 * HS : (h0 + HEADS_PER_CORE) * HS]),
                "bqS": np.ascontiguousarray(
                    bq[h0 * HS : (h0 + HEADS_PER_CORE) * HS].reshape(2, 128).T
                ),
            }
        )

    res = run_bass_kernel_spmd(nc, in_maps, core_ids=list(range(N_CORES)))

    full = np.empty((B, H, S, S), dtype=np.float32)
    for c in range(N_CORES):
        b, hg = divmod(c, N_CORES // B)
        h0 = hg * HEADS_PER_CORE
        full[b, h0 : h0 + HEADS_PER_CORE] = res.results[c]["out"]
    return full


# revision 9
# speedup vs baseline: 1.1343x; 1.1343x over previous
"""Self-attention scores kernel for Trainium2, 8-core SPMD.

Computes softmax((x@Wq+bq) @ (x@Wq+bq)^T / sqrt(64)) per head
(reference reuses the query projection for k, bug-for-bug).

Sharding: 32 (batch, head) pairs split 4-per-core across 8 cores.
Core c handles batch c//4, heads 4*(c%4) .. 4*(c%4)+3.
Each core gets x[b]^T (host-transposed), its Wq column slice, and its
bias slice; it computes q^T = Wq_slice^T @ x^T (+bias), then per head
the [2048, 2048] score block + row softmax, streaming 1 MiB row-blocks
back to HBM.
"""

import numpy as np

import concourse.bass as bass
import concourse.mybir as mybir
import concourse.tile as tile
from concourse import bacc
from concourse.bass_utils import run_bass_kernel_spmd

B = 2
S = 2048
D = 1024
H = 16
HS = 64
N_CORES = 8
HEADS_PER_CORE = 4  # 2 pairs of 2 heads (pair = 128 partitions)
KK = D // 128  # 8 k-tiles for the projection contraction
NQ = S // 128  # 16 q row-blocks per head
NC_ = S // 512  # 4 key chunks of 512

# Matmul input dtype: float32 is exact but 4 cycles/row on the PE;
# float32r runs at full rate for N>=256 with relaxed (tf32-like)
# precision. The BIR verifier requires fp32r matmul operands to be
# *produced* as fp32r, so the input DRAM tensors and SBUF tiles feeding
# the PE are declared float32r (numpy binding is still float32).
MM_DT = mybir.dt.float32r

F32 = mybir.dt.float32


def _mm_view(ap):
    return ap


def _build():
    nc = bacc.Bacc("TRN2", target_bir_lowering=False, debug=False)
    xT = nc.dram_tensor("xT", [D, S], MM_DT, kind="ExternalInput").ap()
    WqS = nc.dram_tensor("WqS", [D, HEADS_PER_CORE * HS], MM_DT, kind="ExternalInput").ap()
    bqS = nc.dram_tensor("bqS", [128, 2], F32, kind="ExternalInput").ap()
    out = nc.dram_tensor("out", [HEADS_PER_CORE, S, S], F32, kind="ExternalOutput").ap()

    with tile.TileContext(nc) as tc:
        with (
            tc.tile_pool(name="consts", bufs=1) as consts,
            tc.tile_pool(name="qt", bufs=2) as qt_pool,
            tc.tile_pool(name="xt", bufs=KK) as xt_pool,
            tc.tile_pool(name="ps_proj", bufs=2, space="PSUM") as ps_proj,
            tc.tile_pool(name="ps_sc", bufs=3, space="PSUM") as ps_sc,
            tc.tile_pool(name="et", bufs=6) as et_pool,
            tc.tile_pool(name="small", bufs=8) as small,
        ):
            w = consts.tile([128, KK, HEADS_PER_CORE * HS], MM_DT)
            nc.sync.dma_start(out=w[:], in_=WqS.rearrange("(kk p) c -> p kk c", p=128))
            bias = consts.tile([128, 2], F32)
            nc.sync.dma_start(out=bias[:], in_=bqS)

            # x^T streamed as 8 independent k-tiles so projection matmuls
            # can start as soon as each tile lands.
            xts = []
            for kk in range(KK):
                xtt = xt_pool.tile([128, S], MM_DT, tag="xt")
                nc.sync.dma_start(out=xtt[:], in_=xT[kk * 128 : (kk + 1) * 128, :])
                xts.append(xtt)

            # ---- Projection for one head-pair ----
            def project(g):
                qtg = qt_pool.tile([128, S], MM_DT, tag="qt")
                for n in range(NC_):
                    ps = ps_proj.tile([128, 512], F32, tag="pp")
                    for kk in range(KK):
                        nc.tensor.matmul(
                            ps[:],
                            lhsT=w[:, kk, g * 128 : (g + 1) * 128],
                            rhs=xts[kk][:, n * 512 : (n + 1) * 512],
                            start=(kk == 0),
                            stop=(kk == KK - 1),
                        )
                    nc.scalar.activation(
                        out=qtg[:, n * 512 : (n + 1) * 512],
                        in_=ps[:],
                        func=mybir.ActivationFunctionType.Identity,
                        bias=bias[:, g : g + 1],
                        scale=1.0,
                    )
                return qtg

            # ---- Scores + softmax for one head, streamed per row-block ----
            def score_head(h, qtg):
                pb = (h % 2) * 64
                for i in range(NQ):
                    lhsT = qtg[pb : pb + 64, i * 128 : (i + 1) * 128]
                    et = et_pool.tile([128, S], F32, tag="et")
                    sums2 = small.tile([128, 2], F32, tag="sm")
                    for half in range(2):
                        ps = ps_sc.tile([128, 1024], F32, tag="ps")
                        for j in (2 * half, 2 * half + 1):
                            nc.tensor.matmul(
                                ps[:, (j % 2) * 512 : (j % 2 + 1) * 512],
                                lhsT=lhsT,
                                rhs=qtg[pb : pb + 64, j * 512 : (j + 1) * 512],
                                start=True,
                                stop=True,
                            )
                        nc.scalar.activation(
                            out=et[:, half * 1024 : (half + 1) * 1024],
                            in_=ps[:],
                            func=mybir.ActivationFunctionType.Exp,
                            scale=1.0 / np.sqrt(float(HS)),
                            accum_out=sums2[:, half : half + 1],
                        )
                    recip = small.tile([128, 1], F32, tag="rc")
                    nc.vector.tensor_add(recip[:], sums2[:, 0:1], sums2[:, 1:2])
                    nc.vector.reciprocal(recip[:], recip[:])
                    nc.vector.tensor_scalar_mul(et[:], et[:], recip[:])
                    nc.sync.dma_start(
                        out=out[h, i * 128 : (i + 1) * 128, :], in_=et[:]
                    )

            # Emission order sets Tile's scheduling priority: get pair-0's
            # output stream going first; pair-1's projection then fills PE
            # idle slots during streaming.
            qt0 = project(0)
            score_head(0, qt0)
            score_head(1, qt0)
            qt1 = project(1)
            score_head(2, qt1)
            score_head(3, qt1)
    nc.compile()
    return nc


_NC_CACHE = None


def kernel(x, Wq, bq):
    global _NC_CACHE
    x = np.asarray(x, dtype=np.float32)
    Wq = np.asarray(Wq, dtype=np.float32)
    bq = np.asarray(bq, dtype=np.float32)
    assert x.shape == (B, S, D) and Wq.shape == (D, D) and bq.shape == (D,)

    if _NC_CACHE is None:
        _NC_CACHE = _build()
    nc = _NC_CACHE

    xTs = [np.ascontiguousarray(x[b].T) for b in range(B)]
    in_maps = []
    for c in range(N_CORES):
        b, hg = divmod(c, N_CORES // B)
        h0 = hg * HEADS_PER_CORE
        in_maps.append(
            {
                "xT": xTs[b],
                "WqS": np.ascontiguousarray(Wq[:, h0 * HS : (h0 + HEADS_PER_CORE) * HS]),
                "bqS": np.ascontiguousarray(
                    bq[h0 * HS : (h0 + HEADS_PER_CORE) * HS].reshape(2, 128).T
                ),
            }
        )

    res = run_bass_kernel_spmd(nc, in_maps, core_ids=list(range(N_CORES)))

    full = np.empty((B, H, S, S), dtype=np.float32)
    for c in range(N_CORES):
        b, hg = divmod(c, N_CORES // B)
        h0 = hg * HEADS_PER_CORE
        full[b, h0 : h0 + HEADS_PER_CORE] = res.results[c]["out"]
    return full
